# revision 1
# baseline (speedup 1.0000x reference)
"""CrystalGraphAttention Trainium2 kernel.

Data-parallel over batch: core b handles batch b (B=8, 8 cores).
Per-core algorithm (all "T" tensors live transposed, [feature, node]):
  xT = x^T                                  (PE transpose)
  qT = (Wq/8)^T xT, kT = (Wk^T xT) * dw_t   (fp32r matmuls; dw folded into K)
  v  = x Wv  -> vaug[t, h, 0:64]=v, [.,.,64]=1  (bf16, ones row for denominator)
  logitsT[t,s] = kT_h^T qT_h                (fp32r, head pairs row-packed)
  ex = exp(logitsT)  (bf16)                 (no max-sub needed: |logit|<~8)
  exm = ex * tfac    where tfac[t,s] = m^T*(1-e_t)+e_t, e_t = exp(-1e9*dw_t)
      (exactly reproduces the (1-m)*NEG additive mask after the dw multiply)
  psum_o[0:64] = vaug^T exm (accum over t), psum_o[64] = denominator
  oT = psum_o[0:64] / den   (reciprocal + gpsimd partition broadcast)
  out = oT^T Wo + bo        (fp32r, bias via ones-row K=1 accumulation)
"""
import sys

if '/opt/trn_rl_repo' not in sys.path:
    sys.path.insert(0, '/opt/trn_rl_repo')

import os

import numpy as np

B, N, D = 8, 1024, 256
H, DK, DV = 8, 64, 64
NEG = -1.0e9
NCORES = 8

_COMPILED = {}


def _build():
    import concourse.bass as bass
    import concourse.mybir as mybir
    import concourse.tile as tile
    from concourse import bacc
    from concourse.masks import make_identity

    f32 = mybir.dt.float32
    f32r = mybir.dt.float32r
    bf16 = mybir.dt.bfloat16
    MULT = mybir.AluOpType.mult
    ADD = mybir.AluOpType.add
    EXP = mybir.ActivationFunctionType.Exp

    nc = bacc.Bacc(None, target_bir_lowering=False)

    x_d = nc.dram_tensor("x", [N, D], f32, kind="ExternalInput")
    m_d = nc.dram_tensor("m", [N, N], f32, kind="ExternalInput")
    dwr_d = nc.dram_tensor("dwrow", [1, N], f32, kind="ExternalInput")
    dwc_d = nc.dram_tensor("dwcol", [128, 8], f32, kind="ExternalInput")
    wq_d = nc.dram_tensor("wq", [D, H * DK], f32, kind="ExternalInput")
    wk_d = nc.dram_tensor("wk", [D, H * DK], f32, kind="ExternalInput")
    wv_d = nc.dram_tensor("wv", [D, H * DV], f32, kind="ExternalInput")
    wo_d = nc.dram_tensor("wo", [H * DV, D], f32, kind="ExternalInput")
    bo_d = nc.dram_tensor("bo", [1, D], f32, kind="ExternalInput")
    out_d = nc.dram_tensor("out", [N, D], f32, kind="ExternalOutput")
    dbg = os.environ.get("KDBG", "0") == "1"
    if dbg:
        dwB_d = nc.dram_tensor("dbg_dwB", [128, N], f32, kind="ExternalOutput")
        kT_d = nc.dram_tensor("dbg_kT", [128, 4 * N], f32, kind="ExternalOutput")
        qT_d = nc.dram_tensor("dbg_qT", [128, 4 * N], f32, kind="ExternalOutput")
        tf_d = nc.dram_tensor("dbg_tfac", [128, 8 * N], mybir.dt.bfloat16, kind="ExternalOutput")
        oT_d = nc.dram_tensor("dbg_oT", [128, 4 * N], f32, kind="ExternalOutput")
        va_d = nc.dram_tensor("dbg_vaug", [128, 4160], mybir.dt.bfloat16, kind="ExternalOutput")
        l0_d = nc.dram_tensor("dbg_l0", [128, 1024], f32, kind="ExternalOutput")
        ex_d = nc.dram_tensor("dbg_ex", [128, 1024], mybir.dt.bfloat16, kind="ExternalOutput")
        exm_d = nc.dram_tensor("dbg_exm", [128, 1024], mybir.dt.bfloat16, kind="ExternalOutput")
        po_d = nc.dram_tensor("dbg_po", [128, 512], f32, kind="ExternalOutput")
        rb_d = nc.dram_tensor("dbg_rb", [128, 512], f32, kind="ExternalOutput")

    with tile.TileContext(nc) as tc:
        with tc.tile_pool(name="const", bufs=1) as cst, \
             tc.tile_pool(name="big", bufs=1) as big, \
             tc.tile_pool(name="xst", bufs=2) as xpool, \
             tc.tile_pool(name="wst", bufs=2) as wpool, \
             tc.tile_pool(name="mst", bufs=5) as mpool, \
             tc.tile_pool(name="exp", bufs=3) as expool, \
             tc.tile_pool(name="exm", bufs=3) as exmpool, \
             tc.tile_pool(name="nrm", bufs=4) as npool, \
             tc.tile_pool(name="outp", bufs=3) as opool, \
             tc.tile_pool(name="psl", bufs=2, space="PSUM") as ps_l, \
             tc.tile_pool(name="pso", bufs=3, space="PSUM") as ps_o, \
             tc.tile_pool(name="psp", bufs=1, space="PSUM") as ps_p:

            ident = cst.tile([128, 128], f32)
            make_identity(nc, ident)

            ones_f = cst.tile([1, 128], f32)
            nc.vector.memset(ones_f, 1.0)
            ones_r = cst.tile([1, 128], f32r)
            nc.vector.tensor_copy(ones_r, ones_f)

            # distance weights: column form [p, tc] and broadcast row form
            dwc = cst.tile([128, 8], f32)
            nc.sync.dma_start(dwc, dwc_d[:, :])
            e_sb = cst.tile([128, 8], f32)
            nc.scalar.activation(e_sb, dwc, EXP, scale=NEG)
            ome = cst.tile([128, 8], f32)
            nc.vector.tensor_scalar(ome, e_sb, -1.0, 1.0, MULT, ADD)
            dwrow = cst.tile([1, N], f32)
            nc.sync.dma_start(dwrow, dwr_d[:, :])
            dwB = cst.tile([128, N], f32)
            nc.gpsimd.partition_broadcast(dwB, dwrow)

            # weights -> fp32r (q pre-scaled by 1/8)
            def load_w(dram, scale):
                st = wpool.tile([128, 1024], f32, tag="wst")
                nc.sync.dma_start(st[:, 0:512], dram[0:128, :])
                nc.sync.dma_start(st[:, 512:1024], dram[128:256, :])
                r = big.tile([128, 1024], f32r, name=dram.name + "_r")
                if scale is None:
                    nc.vector.tensor_copy(r, st)
                else:
                    nc.vector.tensor_scalar_mul(r, st, scale)
                return r

            wq_r = load_w(wq_d, 0.125)
            wk_r = load_w(wk_d, None)
            wv_r = load_w(wv_d, None)
            wo_st = wpool.tile([128, 1024], f32, tag="wst")
            for cc in range(4):
                nc.sync.dma_start(wo_st[:, cc * 256:(cc + 1) * 256],
                                  wo_d[cc * 128:(cc + 1) * 128, :])
            wo_r = big.tile([128, 1024], f32r)
            nc.vector.tensor_copy(wo_r, wo_st)
            bo_f = cst.tile([1, 256], f32)
            nc.sync.dma_start(bo_f, bo_d[:, :])
            bo_r = cst.tile([1, 256], f32r)
            nc.vector.tensor_copy(bo_r, bo_f)

            # ---- xT via PE transpose ----
            xT = big.tile([128, 2 * N], f32r)  # [p=d%128, kd*1024 + n]
            for g in range(2):
                pst0 = ps_l.tile([128, 512], f32, tag="psl")
                pst1 = ps_l.tile([128, 512], f32, tag="psl")
                for i in range(4):
                    nch = g * 4 + i
                    xch = xpool.tile([128, D], f32, tag="xst")
                    nc.sync.dma_start(xch, x_d[nch * 128:(nch + 1) * 128, :])
                    nc.tensor.transpose(pst0[:, i * 128:(i + 1) * 128],
                                        xch[:, 0:128], ident)
                    nc.tensor.transpose(pst1[:, i * 128:(i + 1) * 128],
                                        xch[:, 128:256], ident)
                nc.vector.tensor_copy(xT[:, g * 512:g * 512 + 512], pst0)
                nc.vector.tensor_copy(xT[:, N + g * 512:N + g * 512 + 512], pst1)

            # ---- qT, kT (kT gets dw folded) ----
            qT = big.tile([128, 4 * N], f32r)  # [p, hc*1024 + n]
            kT = big.tile([128, 4 * N], f32r)
            for c4 in range(4):
                for nt in range(2):
                    off = c4 * N + nt * 512
                    psq = ps_l.tile([128, 512], f32, tag="psl")
                    psk = ps_l.tile([128, 512], f32, tag="psl")
                    for kd in range(2):
                        nc.tensor.matmul(
                            psq, wq_r[:, kd * 512 + c4 * 128:kd * 512 + (c4 + 1) * 128],
                            xT[:, kd * N + nt * 512:kd * N + nt * 512 + 512],
                            start=(kd == 0), stop=(kd == 1))
                    for kd in range(2):
                        nc.tensor.matmul(
                            psk, wk_r[:, kd * 512 + c4 * 128:kd * 512 + (c4 + 1) * 128],
                            xT[:, kd * N + nt * 512:kd * N + nt * 512 + 512],
                            start=(kd == 0), stop=(kd == 1))
                    nc.scalar.copy(qT[:, off:off + 512], psq)
                    nc.vector.tensor_tensor(kT[:, off:off + 512], psk,
                                            dwB[:, nt * 512:nt * 512 + 512], MULT)

            # ---- v -> vaug (bf16, ones column at slot 64) ----
            vaug = big.tile([128, 8 * 8 * 65], bf16)  # [p=t%128, tc*520 + h*65 + s]
            vaug4 = vaug.rearrange("p (t h s) -> p t h s", t=8, h=8)
            ones_c = cst.tile([128, 64], f32)
            nc.vector.memset(ones_c, 1.0)
            ones_bf = cst.tile([128, 1], bf16)
            nc.vector.tensor_copy(ones_bf, ones_c[:, 0:1])
            nc.vector.tensor_copy(
                vaug4[:, :, :, 64:65],
                ones_c.rearrange("p (t h o) -> p t h o", t=8, h=8))
            for t8 in range(8):
                psv = ps_l.tile([128, 512], f32, tag="psl")
                for kd in range(2):
                    nc.tensor.matmul(
                        psv, xT[:, kd * N + t8 * 128:kd * N + (t8 + 1) * 128],
                        wv_r[:, kd * 512:(kd + 1) * 512],
                        start=(kd == 0), stop=(kd == 1))
                nc.vector.tensor_copy(
                    vaug4[:, t8:t8 + 1, :, 0:64],
                    psv.rearrange("p (o h s) -> p o h s", o=1, h=8))

            # ---- mask -> tfac (transposed, bf16) ----
            tfac = big.tile([128, 8 * N], bf16)  # [p=t%128, tc*1024 + s]
            tfac3 = tfac.rearrange("p (t s) -> p t s", t=8)
            for g in range(2):
                mchs = []
                for i in range(4):
                    mch = mpool.tile([128, N], f32, tag="mst")
                    sc = g * 4 + i
                    nc.sync.dma_start(mch, m_d[sc * 128:(sc + 1) * 128, :])
                    mchs.append(mch)
                for t8 in range(8):
                    mt = ps_l.tile([128, 512], f32, tag="psl")
                    for i in range(4):
                        nc.tensor.transpose(mt[:, i * 128:(i + 1) * 128],
                                            mchs[i][:, t8 * 128:(t8 + 1) * 128],
                                            ident)
                    nc.vector.tensor_scalar(
                        tfac[:, t8 * N + g * 512:t8 * N + g * 512 + 512],
                        mt, ome[:, t8:t8 + 1], e_sb[:, t8:t8 + 1], MULT, ADD)

            # ---- attention ----
            oT = big.tile([128, 4 * N], f32r)  # [p=hdv%128, cc*1024 + s]
            for st in range(2):
                for c in range(4):
                    h0, h1 = 2 * c, 2 * c + 1
                    po = [ps_o.tile([128, 512], f32, tag="pso", name=f"po_{st}_{c}_{i}") for i in range(2)]
                    for tcp in range(4):
                        tc0, tc1 = 2 * tcp, 2 * tcp + 1
                        ls = [ps_l.tile([128, 1024], f32, tag="psl", name=f"l_{st}_{c}_{tcp}_{i}")
                              for i in range(2)]
                        for tci, t8 in enumerate((tc0, tc1)):
                            for hi, h in enumerate((h0, h1)):
                                p0 = (h % 2) * 64
                                nc.tensor.matmul(
                                    ls[hi][:, tci * 512:tci * 512 + 512],
                                    kT[p0:p0 + 64, c * N + t8 * 128:c * N + (t8 + 1) * 128],
                                    qT[p0:p0 + 64, c * N + st * 512:c * N + st * 512 + 512],
                                    start=True, stop=True)
                        for hi, h in enumerate((h0, h1)):
                            ex = expool.tile([128, 1024], bf16, tag="exp")
                            nc.scalar.activation(ex, ls[hi], EXP)
                            exm = exmpool.tile([128, 1024], bf16, tag="exm")
                            if dbg and st == 0 and c == 0 and tcp == 0 and hi == 0:
                                lcp = big.tile([128, 1024], f32, name="lcp")
                                nc.vector.tensor_copy(lcp, ls[hi])
                                nc.sync.dma_start(l0_d[:, :], lcp)
                                nc.sync.dma_start(ex_d[:, :], ex)
                            nc.vector.tensor_tensor(
                                exm.rearrange("p (t s) -> p t s", t=2),
                                ex.rearrange("p (t s) -> p t s", t=2),
                                tfac3[:, tc0:tc0 + 2, st * 512:st * 512 + 512],
                                MULT)
                            if dbg and st == 0 and c == 0 and tcp == 0 and hi == 0:
                                nc.sync.dma_start(exm_d[:, :], exm)
                            for tci, t8 in enumerate((tc0, tc1)):
                                first = (tcp == 0 and tci == 0)
                                last = (tcp == 3 and tci == 1)
                                vo = t8 * 520 + h * 65
                                if hi == 0:
                                    nc.tensor.matmul(
                                        po[hi][0:65, :],
                                        vaug[:, vo:vo + 65],
                                        exm[:, tci * 512:tci * 512 + 512],
                                        start=first, stop=last)
                                else:
                                    nc.tensor.matmul(
                                        po[hi][0:1, :], ones_bf,
                                        exm[:, tci * 512:tci * 512 + 512],
                                        start=first, stop=last)
                                    nc.tensor.matmul(
                                        po[hi][64:128, :],
                                        vaug[:, vo:vo + 64],
                                        exm[:, tci * 512:tci * 512 + 512],
                                        start=first, stop=last)
                    for hi, h in enumerate((h0, h1)):
                        p0 = (h % 2) * 64
                        if dbg and st == 0 and c == 0 and hi == 0:
                            pcp = big.tile([128, 512], f32, name="pcp")
                            nc.vector.tensor_copy(pcp, po[hi])
                            nc.sync.dma_start(po_d[:, :], pcp)
                        dtmp = npool.tile([128, 512], f32, tag="nrmt")
                        if hi == 0:
                            nc.vector.tensor_copy(dtmp[64:65, :], po[hi][64:65, :])
                            den0 = npool.tile([1, 512], f32, tag="nrm")
                            nc.sync.dma_start(den0, dtmp[64:65, :])
                            bsrc = den0
                        else:
                            nc.vector.tensor_copy(dtmp[0:1, :], po[hi][0:1, :])
                            bsrc = dtmp[0:1, :]
                        den_b = npool.tile([128, 512], f32, tag="nrmb")
                        nc.gpsimd.partition_broadcast(den_b, bsrc)
                        rb = npool.tile([128, 512], f32, tag="nrmc")
                        nc.vector.reciprocal_approx_fast(rb, den_b)
                        if dbg and st == 0 and c == 0 and hi == 0:
                            nc.sync.dma_start(rb_d[:, :], rb)
                        nc.vector.tensor_tensor(
                            oT[p0:p0 + 64, c * N + st * 512:c * N + st * 512 + 512],
                            po[hi][p0:p0 + 64, :], rb[p0:p0 + 64, :], MULT)

            if dbg:
                nc.sync.dma_start(dwB_d[:, :], dwB)
                nc.sync.dma_start(kT_d[:, :], kT.bitcast(f32))
                nc.sync.dma_start(qT_d[:, :], qT.bitcast(f32))
                nc.sync.dma_start(tf_d[:, :], tfac)
                nc.sync.dma_start(oT_d[:, :], oT.bitcast(f32))
                nc.sync.dma_start(va_d[:, :], vaug)

            # ---- output projection + bias ----
            for sc in range(8):
                psp = ps_p.tile([128, 256], f32, tag="psp")
                for cc in range(4):
                    nc.tensor.matmul(
                        psp, oT[:, cc * N + sc * 128:cc * N + (sc + 1) * 128],
                        wo_r[:, cc * 256:(cc + 1) * 256],
                        start=(cc == 0), stop=False)
                nc.tensor.matmul(psp, ones_r[0:1, :], bo_r[0:1, :],
                                 start=False, stop=True)
                ot = opool.tile([128, 256], f32, tag="outp")
                nc.scalar.copy(ot, psp)
                nc.sync.dma_start(out_d[sc * 128:(sc + 1) * 128, :], ot)

    nc.compile()
    return nc


def _get_compiled():
    if 'nc' not in _COMPILED:
        _COMPILED['nc'] = _build()
    return _COMPILED['nc']


def _shard(inputs):
    x = np.ascontiguousarray(inputs['node_features'], dtype=np.float32)
    em = np.ascontiguousarray(inputs['edge_mask'], dtype=np.float32)
    dw = np.ascontiguousarray(inputs['distance_weights'], dtype=np.float32)
    wq = np.ascontiguousarray(inputs['Wq'], dtype=np.float32)
    wk = np.ascontiguousarray(inputs['Wk'], dtype=np.float32)
    wv = np.ascontiguousarray(inputs['Wv'], dtype=np.float32)
    wo = np.ascontiguousarray(inputs['Wo'], dtype=np.float32)
    bo = np.ascontiguousarray(inputs['bo'], dtype=np.float32).reshape(1, D)
    maps = []
    for b in range(NCORES):
        maps.append({
            "x": x[b],
            "m": np.ascontiguousarray(em[b, 0]),
            "dwrow": np.ascontiguousarray(dw[b].reshape(1, N)),
            "dwcol": np.ascontiguousarray(dw[b].reshape(8, 128).T),
            "wq": wq, "wk": wk, "wv": wv, "wo": wo, "bo": bo,
        })
    return maps


def run_sharded(inputs, **kwargs):
    from concourse.bass_utils import run_bass_kernel_spmd
    nc = _get_compiled()
    maps = _shard(inputs)
    res = run_bass_kernel_spmd(nc, maps, core_ids=list(range(NCORES)), **kwargs)
    out = np.stack([res.results[b]["out"] for b in range(NCORES)], axis=0)
    return out, res


def kernel(**inputs) -> np.ndarray:
    out, _ = run_sharded(inputs)
    return out



# revision 8
# speedup vs baseline: 1.3139x; 1.3139x over previous
"""CrystalGraphAttention Trainium2 kernel (v2).

Data-parallel over batch: core b handles batch b (B=8, 8 cores).

Per-core design (everything transposed as [feature, node] where useful):
  xT   [128, kd(2)*1024] bf16     via XBAR DMA transpose (u16 hi-half trick)
  mT   [128, t8(8)*1024] bf16     via XBAR DMA transpose (exact for 0/1 mask)
  qT/kT[128, c4(4)*1024] bf16     heads packed pairwise: head h ->
                                  partitions (h%2)*64..+64, block c=h//2
  kT has dw (per-key distance weight) folded in; the 1/8 (=1/sqrt(dk))
  scale is folded into the exp activation's scale argument.
  tfac [128, t8(8)*1024] bf16     = mT*(1-e_t)+e_t, e_t=exp(NEG*dw_t):
                                  multiplicative equivalent of the additive
                                  (1-m)*NEG mask after the dw multiply.
  vaug [128, t8(8)*h(8)*65] bf16  V plus a ones column per head: the 65-wide
                                  AV matmul output carries the softmax
                                  denominator at PSUM partition 64 for free.
  AV accumulates po[0:65, s] over t8; normalization divides by the broadcast
  reciprocal of the den row; odd heads reach oT partitions 64:128 via a
  partition-shifting SBUF->SBUF DMA (DVE lanes cannot shift partitions).
  out = oT^T Wo + bo (bias via ones-row K=1 matmul into the same PSUM group).
"""
import os
import sys

if '/opt/trn_rl_repo' not in sys.path:
    sys.path.insert(0, '/opt/trn_rl_repo')

import numpy as np

B, N, D = 8, 1024, 256
H, DK, DV = 8, 64, 64
NEG = -1.0e9
NCORES = 8

_COMPILED = {}


def _build():
    import concourse.bass as bass
    import concourse.mybir as mybir
    import concourse.tile as tile
    from concourse import bacc

    f32 = mybir.dt.float32
    f32r = mybir.dt.float32r
    bf16 = mybir.dt.bfloat16
    u16 = mybir.dt.uint16
    MULT = mybir.AluOpType.mult
    ADD = mybir.AluOpType.add
    EXP = mybir.ActivationFunctionType.Exp

    nc = bacc.Bacc(None, target_bir_lowering=False)

    # f32 inputs exposed as uint16 so strided DMA grabs the bf16 hi-halves
    x_d = nc.dram_tensor("x", [N, D], u16, kind="ExternalInput")
    m_d = nc.dram_tensor("m", [N, N], u16, kind="ExternalInput")
    dwr_d = nc.dram_tensor("dwrow", [1, N], f32, kind="ExternalInput")
    dwc_d = nc.dram_tensor("dwcol", [128, 8], f32, kind="ExternalInput")
    wq_d = nc.dram_tensor("wq", [D, H * DK], u16, kind="ExternalInput")
    wk_d = nc.dram_tensor("wk", [D, H * DK], u16, kind="ExternalInput")
    wv_d = nc.dram_tensor("wv", [D, H * DV], u16, kind="ExternalInput")
    wo_d = nc.dram_tensor("wo", [H * DV, D], u16, kind="ExternalInput")
    bo_d = nc.dram_tensor("bo", [1, D], f32, kind="ExternalInput")
    out_d = nc.dram_tensor("out", [N, D], f32, kind="ExternalOutput")
    dbg = os.environ.get("KDBG", "0") == "1"
    if dbg:
        dxT = nc.dram_tensor("dbg_xT", [128, 2 * N], mybir.dt.bfloat16, kind="ExternalOutput")
        dmT = nc.dram_tensor("dbg_mT", [128, 8 * N], mybir.dt.bfloat16, kind="ExternalOutput")
        dtf = nc.dram_tensor("dbg_tfac", [128, 8 * N], mybir.dt.bfloat16, kind="ExternalOutput")
        dqT = nc.dram_tensor("dbg_qT", [128, 4 * N], mybir.dt.bfloat16, kind="ExternalOutput")
        dkT = nc.dram_tensor("dbg_kT", [128, 4 * N], mybir.dt.bfloat16, kind="ExternalOutput")
        dva = nc.dram_tensor("dbg_vaug", [128, 4160], mybir.dt.bfloat16, kind="ExternalOutput")
        doT = nc.dram_tensor("dbg_oT", [128, 4 * N], mybir.dt.bfloat16, kind="ExternalOutput")
        dex = nc.dram_tensor("dbg_ex", [128, 2 * N], mybir.dt.bfloat16, kind="ExternalOutput")
        dexm = nc.dram_tensor("dbg_exm", [128, 2 * N], mybir.dt.bfloat16, kind="ExternalOutput")
        ddr = nc.dram_tensor("dbg_dr", [1, N], f32, kind="ExternalOutput")

    with tile.TileContext(nc) as tc:
        with tc.tile_pool(name="const", bufs=1) as cst, \
             tc.tile_pool(name="big", bufs=1) as big, \
             tc.tile_pool(name="exp", bufs=2) as expool, \
             tc.tile_pool(name="exm", bufs=3) as exmpool, \
             tc.tile_pool(name="dsb", bufs=2) as dpool, \
             tc.tile_pool(name="drr", bufs=2) as drpool, \
             tc.tile_pool(name="rbp", bufs=2) as rbpool, \
             tc.tile_pool(name="otm", bufs=2) as otmpool, \
             tc.tile_pool(name="outp", bufs=3) as opool, \
             tc.tile_pool(name="psl", bufs=2, space="PSUM") as ps_l, \
             tc.tile_pool(name="pso", bufs=2, space="PSUM") as ps_o:

            # ---- distance weights ----
            dwc = cst.tile([128, 8], f32)
            nc.sync.dma_start(dwc, dwc_d[:, :])
            e_sb = cst.tile([128, 8], f32)
            nc.scalar.activation(e_sb, dwc, EXP, scale=NEG)
            ome = cst.tile([128, 8], f32)
            nc.vector.tensor_scalar(ome, e_sb, -1.0, 1.0, MULT, ADD)
            dwrow = cst.tile([1, N], f32)
            nc.sync.dma_start(dwrow, dwr_d[:, :])
            dwB = cst.tile([128, N], f32)
            nc.gpsimd.partition_broadcast(dwB, dwrow)

            ones_f = cst.tile([1, 128], f32)
            nc.vector.memset(ones_f, 1.0)
            ones_r = cst.tile([1, 128], f32r)
            nc.vector.tensor_copy(ones_r, ones_f)
            bo_f = cst.tile([1, 256], f32)
            nc.sync.dma_start(bo_f, bo_d[:, :])
            bo_r = cst.tile([1, 256], f32r)
            nc.vector.tensor_copy(bo_r, bo_f)

            # ---- xT via XBAR DMA transpose (bf16) ----
            xT = big.tile([128, 2 * N], bf16)  # [p=d%128, kd*1024 + n]
            for kd in range(2):
                nc.sync.dma_start(
                    xT[:, kd * N:(kd + 1) * N].bitcast(u16),
                    x_d[:, kd * 128:(kd + 1) * 128],
                    transpose=True)

            # ---- weights (strided bf16 loads) ----
            wq_sb = big.tile([128, 2 * 512], bf16)  # [p=d%128, kd*512 + col]
            wk_sb = big.tile([128, 2 * 512], bf16)
            wv_sb = big.tile([128, 2 * 512], bf16)
            for w_sb, w_d in ((wq_sb, wq_d), (wk_sb, wk_d), (wv_sb, wv_d)):
                for kd in range(2):
                    nc.sync.dma_start(
                        w_sb[:, kd * 512:(kd + 1) * 512].bitcast(u16),
                        w_d[kd * 128:(kd + 1) * 128, :])
            wo_sb = big.tile([128, 4 * 256], bf16)  # [p=hd%128, cc*256 + d]
            for cc in range(4):
                nc.sync.dma_start(
                    wo_sb[:, cc * 256:(cc + 1) * 256].bitcast(u16),
                    wo_d[cc * 128:(cc + 1) * 128, :])

            # ---- mT + tfac (XBAR transpose + fused affine) ----
            mT = big.tile([128, 8 * N], bf16)    # [p=t%128, t8*1024 + s]
            tfac = big.tile([128, 8 * N], bf16)  # [p=t%128, t8*1024 + s]
            for t8 in range(8):
                nc.sync.dma_start(
                    mT[:, t8 * N:(t8 + 1) * N].bitcast(u16),
                    m_d[:, t8 * 128:(t8 + 1) * 128],
                    transpose=True)
                nc.vector.tensor_scalar(
                    tfac[:, t8 * N:(t8 + 1) * N],
                    mT[:, t8 * N:(t8 + 1) * N],
                    ome[:, t8:t8 + 1], e_sb[:, t8:t8 + 1], MULT, ADD)

            # ---- q/k projections: [p=head pair, c4*1024 + n] ----
            qT = big.tile([128, 4 * N], bf16)
            kT = big.tile([128, 4 * N], bf16)
            for c4 in range(4):
                psq = ps_l.tile([128, N], f32, tag="psl")
                psk = ps_o.tile([128, N], f32, tag="pso")
                for kd in range(2):
                    for sh in range(2):
                        nc.tensor.matmul(
                            psq[:, sh * 512:(sh + 1) * 512],
                            wq_sb[:, kd * 512 + c4 * 128:kd * 512 + (c4 + 1) * 128],
                            xT[:, kd * N + sh * 512:kd * N + (sh + 1) * 512],
                            start=(kd == 0), stop=(kd == 1))
                for kd in range(2):
                    for sh in range(2):
                        nc.tensor.matmul(
                            psk[:, sh * 512:(sh + 1) * 512],
                            wk_sb[:, kd * 512 + c4 * 128:kd * 512 + (c4 + 1) * 128],
                            xT[:, kd * N + sh * 512:kd * N + (sh + 1) * 512],
                            start=(kd == 0), stop=(kd == 1))
                nc.scalar.copy(qT[:, c4 * N:(c4 + 1) * N], psq)
                nc.vector.tensor_tensor(kT[:, c4 * N:(c4 + 1) * N], psk,
                                        dwB, MULT)

            # ---- v -> vaug (bf16, ones column at slot 64 per head) ----
            vaug = big.tile([128, 8 * 8 * 65], bf16)  # [p=t%128, t8*520+h*65+c]
            vaug4 = vaug.rearrange("p (t h s) -> p t h s", t=8, h=8)
            nc.vector.memset(vaug4[:, :, :, 64:65], 1.0)
            for t8 in range(8):
                psv = ps_l.tile([128, 512], f32, tag="psl")
                for kd in range(2):
                    nc.tensor.matmul(
                        psv, xT[:, kd * N + t8 * 128:kd * N + (t8 + 1) * 128],
                        wv_sb[:, kd * 512:(kd + 1) * 512],
                        start=(kd == 0), stop=(kd == 1))
                nc.vector.tensor_copy(
                    vaug4[:, t8:t8 + 1, :, 0:64],
                    psv.rearrange("p (o h s) -> p o h s", o=1, h=8))

            # ---- attention ----
            oT = big.tile([128, 4 * N], bf16)  # [p=hdv%128, c*1024 + s]
            for h in range(8):
                c, odd = h // 2, h % 2
                p0 = odd * 64
                po = ps_o.tile([128, N], f32, tag="pso")
                for t4 in range(4):
                    ex = expool.tile([128, 2 * N], bf16, tag="exp")
                    for tci in range(2):
                        t8 = 2 * t4 + tci
                        ls = ps_l.tile([128, N], f32, tag="psl")
                        for sh in range(2):
                            nc.tensor.matmul(
                                ls[:, sh * 512:(sh + 1) * 512],
                                kT[p0:p0 + 64, c * N + t8 * 128:c * N + (t8 + 1) * 128],
                                qT[p0:p0 + 64, c * N + sh * 512:c * N + (sh + 1) * 512],
                                start=True, stop=True)
                        nc.scalar.activation(ex[:, tci * N:(tci + 1) * N],
                                             ls, EXP, scale=0.125)
                    exm = exmpool.tile([128, 2 * N], bf16, tag="exm")
                    nc.vector.tensor_tensor(
                        exm, ex, tfac[:, 2 * t4 * N:2 * (t4 + 1) * N], MULT)
                    if dbg and h == 0 and t4 == 0:
                        nc.sync.dma_start(dex[:, :], ex)
                        nc.sync.dma_start(dexm[:, :], exm)
                    for tci in range(2):
                        t8 = 2 * t4 + tci
                        vo = t8 * 520 + h * 65
                        for sh in range(2):
                            nc.tensor.matmul(
                                po[0:65, sh * 512:(sh + 1) * 512],
                                vaug[:, vo:vo + 65],
                                exm[:, tci * N + sh * 512:tci * N + (sh + 1) * 512],
                                start=(t8 == 0), stop=(t8 == 7))
                # normalize: den row at PSUM partition 64 (copy must be a
                # standard tracked op -- a custom-DVE read of an open PSUM
                # accumulation group races with the accumulating matmuls)
                dsb = dpool.tile([128, N], f32, tag="dsb")
                nc.vector.tensor_copy(dsb[64:65, :], po[64:65, :])
                dr = drpool.tile([1, N], f32, tag="drr")
                nc.sync.dma_start(dr, dsb[64:65, :])
                den_b = rbpool.tile([128, N], f32, tag="rbb")
                nc.gpsimd.partition_broadcast(den_b, dr)
                rb = rbpool.tile([128, N], f32, tag="rbc")
                nc.vector.reciprocal_approx_fast(rb, den_b)
                if dbg and h == 0:
                    nc.sync.dma_start(ddr[:, :], dr)
                if odd == 0:
                    nc.vector.tensor_tensor(
                        oT[0:64, c * N:(c + 1) * N],
                        po[0:64, :], rb[0:64, :], MULT)
                else:
                    otmp = otmpool.tile([64, N], bf16, tag="otm")
                    nc.vector.tensor_tensor(otmp, po[0:64, :], rb[0:64, :],
                                            MULT)
                    nc.sync.dma_start(oT[64:128, c * N:(c + 1) * N], otmp)

            if dbg:
                nc.sync.dma_start(dxT[:, :], xT)
                nc.sync.dma_start(dmT[:, :], mT)
                nc.sync.dma_start(dtf[:, :], tfac)
                nc.sync.dma_start(dqT[:, :], qT)
                nc.sync.dma_start(dkT[:, :], kT)
                nc.sync.dma_start(dva[:, :], vaug)
                nc.sync.dma_start(doT[:, :], oT)

            # ---- output projection + bias ----
            for sc in range(8):
                psp = ps_l.tile([128, 256], f32, tag="psl")
                for cc in range(4):
                    nc.tensor.matmul(
                        psp, oT[:, cc * N + sc * 128:cc * N + (sc + 1) * 128],
                        wo_sb[:, cc * 256:(cc + 1) * 256],
                        start=(cc == 0), stop=False)
                nc.tensor.matmul(psp, ones_r[0:1, :], bo_r[0:1, :],
                                 start=False, stop=True)
                ot = opool.tile([128, 256], f32, tag="outp")
                nc.scalar.copy(ot, psp)
                nc.sync.dma_start(out_d[sc * 128:(sc + 1) * 128, :], ot)

    nc.compile()
    return nc


def _get_compiled():
    if 'nc' not in _COMPILED:
        _COMPILED['nc'] = _build()
    return _COMPILED['nc']


def _bf16(a):
    """f32 array -> bf16 bit pattern (uint16), round-to-nearest-even."""
    b = np.ascontiguousarray(a, dtype=np.float32).view(np.uint32)
    return ((b + 0x7FFF + ((b >> 16) & 1)) >> 16).astype(np.uint16)


def _shard(inputs):
    x = _bf16(inputs['node_features'])
    em = _bf16(inputs['edge_mask'])
    dw = np.ascontiguousarray(inputs['distance_weights'], dtype=np.float32)
    wq = _bf16(inputs['Wq'])
    wk = _bf16(inputs['Wk'])
    wv = _bf16(inputs['Wv'])
    wo = _bf16(inputs['Wo'])
    bo = np.ascontiguousarray(inputs['bo'], dtype=np.float32).reshape(1, D)
    maps = []
    for b in range(NCORES):
        maps.append({
            "x": np.ascontiguousarray(x[b]),
            "m": np.ascontiguousarray(em[b, 0]),
            "dwrow": np.ascontiguousarray(dw[b].reshape(1, N)),
            "dwcol": np.ascontiguousarray(dw[b].reshape(8, 128).T),
            "wq": wq, "wk": wk, "wv": wv, "wo": wo, "bo": bo,
        })
    return maps


def run_sharded(inputs, **kwargs):
    from concourse.bass_utils import run_bass_kernel_spmd
    nc = _get_compiled()
    maps = _shard(inputs)
    res = run_bass_kernel_spmd(nc, maps, core_ids=list(range(NCORES)), **kwargs)
    out = np.stack([res.results[b]["out"] for b in range(NCORES)], axis=0)
    return out, res


def kernel(**inputs) -> np.ndarray:
    out, _ = run_sharded(inputs)
    return out
